# revision 7
# baseline (speedup 1.0000x reference)
"""Trainium2 Bass kernel for nn_NeuralCellularAutomata.

Reference computation (B=32, C=16, H=W=128):
    xsum = x.sum(ch)                                  (B,1,H,W)
    sx = conv(xsum, SOBEL_X) + bias_x[c]              (B,16,H,W)
    sy = conv(xsum, SOBEL_X.T) + bias_y[c]            (B,16,H,W)
    p  = concat([sx, sy, x]) transposed to (B,W,H,48)
    h  = relu(p @ W1.T + b1)                          (B,W,H,128)
    out = h @ W2.T + b2                               (B,W,H,16)

Device algebra (validated vs reference on host, rel err ~3e-7):
    h_pre[o,n] = sum_c W1[o,32+c] x[c,n] + A[o] u[n] + B[o] v[n] + b1'[o]
    with A = W1[:,0:16].sum(1), B = W1[:,16:32].sum(1),
    b1' = b1 + W1[:,0:16]@bias_x + W1[:,16:32]@bias_y,
    u = Tmat @ hdiff (vertical [1,2,1]/8 conv as matmul, horiz diff on DVE)
    v = Dmat @ hsmooth (vertical [-1,0,1]/8 as matmul, horiz [1,2,1] on DVE)

Sharding: pure data-parallel, 4 batches per core across 8 cores.

Per-core dataflow (per batch):
    xh [h,(c,w)] <- DMA; GPSIMD tree-sum -> xsum [h,w] (padded cols)
    DVE horiz diff/smooth; PE matmuls Tmat/Dmat -> u,v in PSUM
    flatten-DMA u,v into rhs rows 16,17 of the channel-major x tile
    MM1 (f32r, 2-way row-tiled K=18 at partitions 0/64) -> h_pre PSUM
    relu+bias: ACT + DVE split, writes h bf16
    MM2 flipped (lhsT=h block bf16, rhs=W2^T) -> out^T [w,(h,o)] PSUM
    DVE +b2 -> SBUF -> contiguous DMA store (no transpose needed)
"""

import numpy as np
import ml_dtypes

B_FULL, C, H, W = 32, 16, 128, 128
N_CORES = 8
B_SHARD = B_FULL // N_CORES  # 4
O1 = 128  # hidden
O2 = 16   # out channels
NSP = H * W  # spatial per batch = 16384


def _make_T(n):
    T = np.zeros((n, n), np.float32)
    for m in range(n):
        for dk, wt in ((-1, 1.0), (0, 2.0), (1, 1.0)):
            k = m + dk
            if 0 <= k < n:
                T[m, k] = wt / 8.0
    return T


def _make_D(n):
    D = np.zeros((n, n), np.float32)
    for m in range(n):
        for dk, wt in ((-1, -1.0), (1, 1.0)):
            k = m + dk
            if 0 <= k < n:
                D[m, k] = wt / 8.0
    return D


def build_module():
    import concourse.bacc as bacc
    import concourse.mybir as mybir
    import concourse.tile as tile

    f32 = mybir.dt.float32
    f32r = mybir.dt.float32r
    bf16 = mybir.dt.bfloat16

    nc = bacc.Bacc("TRN2", target_bir_lowering=False, debug=False)

    x_d = nc.dram_tensor("x", [B_SHARD, C, H, W], f32, kind="ExternalInput").ap()
    xr_d = nc.dram_tensor("xr", [B_SHARD, C, H, W], f32r, kind="ExternalInput").ap()
    wext_d = nc.dram_tensor("wext", [128, 128], f32r, kind="ExternalInput").ap()
    tt_d = nc.dram_tensor("tt", [128, 128], f32, kind="ExternalInput").ap()
    dt_d = nc.dram_tensor("dt", [128, 128], f32, kind="ExternalInput").ap()
    b1p_d = nc.dram_tensor("b1p", [128, 1], f32, kind="ExternalInput").ap()
    w2t_d = nc.dram_tensor("w2t", [128, O2], bf16, kind="ExternalInput").ap()
    b2t_d = nc.dram_tensor("b2t", [128, 512], f32, kind="ExternalInput").ap()
    out_d = nc.dram_tensor("out", [B_SHARD, W, H, O2], f32, kind="ExternalOutput").ap()

    RELU = mybir.ActivationFunctionType.Relu
    ADD = mybir.AluOpType.add
    SUB = mybir.AluOpType.subtract
    MULT = mybir.AluOpType.mult
    MAX = mybir.AluOpType.max

    with tile.TileContext(nc) as tc:
        with (
            tc.tile_pool(name="consts", bufs=1) as pc,
            tc.tile_pool(name="xh", bufs=2) as pxh,
            tc.tile_pool(name="xc", bufs=2) as pxc,
            tc.tile_pool(name="gpt", bufs=2) as pgp,
            tc.tile_pool(name="conv", bufs=2) as pcv,
            tc.tile_pool(name="hbuf", bufs=3) as ph,
            tc.tile_pool(name="outs", bufs=2) as pos,
            tc.tile_pool(name="mm1p", bufs=2, space="PSUM") as pmm1,
            tc.tile_pool(name="uvp", bufs=1, space="PSUM") as puv,
            tc.tile_pool(name="outp", bufs=3, space="PSUM") as pop,
        ):
            # constants
            wext = pc.tile([128, 128], f32r)
            ttm = pc.tile([128, 128], f32)
            dtm = pc.tile([128, 128], f32)
            b1p = pc.tile([128, 1], f32)
            w2t = pc.tile([128, O2], bf16)
            b2t = pc.tile([128, 512], f32)
            nc.sync.dma_start(wext[:, :], wext_d)
            nc.sync.dma_start(ttm[:, :], tt_d)
            nc.sync.dma_start(dtm[:, :], dt_d)
            nc.sync.dma_start(b1p[:, :], b1p_d)
            nc.sync.dma_start(w2t[:, :], w2t_d)
            nc.sync.dma_start(b2t[:, :], b2t_d)

            for b in range(B_SHARD):
                # ---- load x (h-major) and compute xsum via GPSIMD tree ----
                xh = pxh.tile([128, C * W], f32, tag="xh")  # [h, (c,w)]
                nc.sync.dma_start(
                    xh[:, :].rearrange("p (c w) -> p c w", c=C),
                    x_d[b].rearrange("c h w -> h c w"))
                xh3 = xh[:, :].rearrange("p (c w) -> p c w", c=C)

                t1 = pgp.tile([128, 8 * W], f32, tag="t1")
                t13 = t1[:, :].rearrange("p (c w) -> p c w", c=8)
                t2 = pgp.tile([128, 4 * W], f32, tag="t2")
                t23 = t2[:, :].rearrange("p (c w) -> p c w", c=4)
                xsp = pgp.tile([128, W + 2], f32, tag="xsp")  # padded xsum

                nc.gpsimd.tensor_tensor(t1[:, :], xh3[:, 0:8, :], xh3[:, 8:16, :], ADD)
                nc.gpsimd.tensor_tensor(t2[:, :], t13[:, 0:4, :], t13[:, 4:8, :], ADD)
                nc.gpsimd.tensor_tensor(t2[:, 0:2 * W], t23[:, 0:2, :], t23[:, 2:4, :], ADD)
                nc.gpsimd.memset(xsp[:, 0:1], 0.0)
                nc.gpsimd.memset(xsp[:, W + 1:W + 2], 0.0)
                nc.gpsimd.tensor_tensor(xsp[:, 1:W + 1], t2[:, 0:W], t2[:, W:2 * W], ADD)

                # ---- horizontal conv parts on DVE ----
                hdiff = pcv.tile([128, W], f32, tag="hdiff")
                hsm = pcv.tile([128, W], f32, tag="hsm")
                nc.vector.tensor_tensor(hdiff[:, :], xsp[:, 2:W + 2], xsp[:, 0:W], SUB)
                nc.vector.tensor_scalar(hsm[:, :], xsp[:, 1:W + 1], 2.0, None, MULT)
                nc.vector.tensor_tensor(hsm[:, :], hsm[:, :], xsp[:, 0:W], ADD)
                nc.vector.tensor_tensor(hsm[:, :], hsm[:, :], xsp[:, 2:W + 2], ADD)

                # ---- vertical conv parts as PE matmuls -> uv psum ----
                uvp = puv.tile([128, 2 * W], f32, tag="uv")
                nc.tensor.matmul(uvp[:, 0:W], lhsT=ttm[:, :], rhs=hdiff[:, :])
                nc.tensor.matmul(uvp[:, W:2 * W], lhsT=dtm[:, :], rhs=hsm[:, :])
                uvs = pcv.tile([128, 2 * W], f32r, tag="uvs")
                nc.scalar.copy(uvs[:, :], uvp[:, :])

                # ---- channel-major x tile (MM1 rhs), 2 row groups ----
                # partition 64*G + c holds x[b, c, 64G:64G+64, :]  (16KB... 64*128*4B = 32KB)
                xc = pxc.tile([128, 64 * W], f32r, tag="xc")
                nc.sync.dma_start(
                    xc[0:C, :], xr_d[b, :, 0:64, :].rearrange("c h w -> c (h w)"))
                nc.sync.dma_start(
                    xc[64:64 + C, :], xr_d[b, :, 64:128, :].rearrange("c h w -> c (h w)"))
                # flatten u,v into rows 16,17 of each group
                for g in range(2):
                    nc.sync.dma_start(
                        xc[16 + 64 * g:17 + 64 * g, :].rearrange("p (h w) -> p h w", h=64),
                        uvs[64 * g:64 * g + 64, 0:W])
                    nc.sync.dma_start(
                        xc[17 + 64 * g:18 + 64 * g, :].rearrange("p (h w) -> p h w", h=64),
                        uvs[64 * g:64 * g + 64, W:2 * W])

                xcr = xc[:, :]
                wextr = wext[:, :]

                # ---- main pipeline: 8 rounds of N=1024 per group ----
                for half in range(2):
                    opA = pop.tile([128, 512], f32, tag="op")  # G0 h-rows
                    opB = pop.tile([128, 512], f32, tag="op")  # G1 h-rows
                    for rr in range(4):
                        r = half * 4 + rr
                        mmA = pmm1.tile([128, 1024], f32, tag="mm1")
                        mmB = pmm1.tile([128, 1024], f32, tag="mm1")
                        for i in range(2):
                            nc.tensor.matmul(
                                mmA[:, 512 * i:512 * (i + 1)],
                                lhsT=wextr[0:18, :],
                                rhs=xcr[0:18, 1024 * r + 512 * i:1024 * r + 512 * (i + 1)])
                        for i in range(2):
                            nc.tensor.matmul(
                                mmB[:, 512 * i:512 * (i + 1)],
                                lhsT=wextr[64:82, :],
                                rhs=xcr[64:82, 1024 * r + 512 * i:1024 * r + 512 * (i + 1)])

                        # relu + bias -> h (bf16), split ACT/DVE
                        h = ph.tile([128, 2048], bf16, tag="h")
                        nc.scalar.activation(h[:, 0:1024], mmA[:, :], RELU, bias=b1p[:, :])
                        nc.scalar.activation(h[:, 1024:1280], mmB[:, 0:256], RELU, bias=b1p[:, :])
                        nc.vector.tensor_scalar(
                            h[:, 1280:2048], mmB[:, 256:1024], b1p[:, :], 0.0, ADD, MAX)

                        # flipped MM2: 16 blocks of 128 spatial
                        for q in range(16):
                            g = q // 8
                            qq = q % 8
                            hrow = 64 * g + 8 * r + qq
                            op = opA if g == 0 else opB
                            nc.tensor.matmul(
                                op[:, (hrow % 32) * O2:(hrow % 32 + 1) * O2],
                                lhsT=h[:, 1024 * g + 128 * qq:1024 * g + 128 * (qq + 1)],
                                rhs=w2t[:, :])

                    # drain out psum: +b2 -> out_sbuf
                    if half == 0:
                        outs = pos.tile([128, H * O2], f32, tag="outs")
                    # h-row ranges: opA -> [32*half, 32*half+32), opB -> 64+that
                    blkA = half
                    blkB = 2 + half
                    nc.vector.tensor_tensor(
                        outs[:, blkA * 512:(blkA + 1) * 512], opA[:, :], b2t[:, :], ADD)
                    nc.vector.tensor_tensor(
                        outs[:, blkB * 512:(blkB + 1) * 512], opB[:, :], b2t[:, :], ADD)

                # ---- store: out_sbuf [w, (h,o)] -> DRAM contiguous ----
                nc.sync.dma_start(out_d[b].rearrange("w h o -> w (h o)"), outs[:, :])

    nc.compile()
    return nc


def make_const_inputs(bias_x, bias_y, W1, b1, W2, b2):
    W1 = np.asarray(W1, np.float32)
    A = W1[:, 0:16].sum(axis=1)
    Bv = W1[:, 16:32].sum(axis=1)
    W1x = W1[:, 32:48]
    b1p = (np.asarray(b1, np.float32)
           + W1[:, 0:16] @ np.asarray(bias_x, np.float32)
           + W1[:, 16:32] @ np.asarray(bias_y, np.float32))

    wext = np.zeros((128, 128), np.float32)
    for g in range(2):
        base = 64 * g
        wext[base:base + 16, :] = W1x.T  # row c -> W1x[:, c]
        wext[base + 16, :] = A
        wext[base + 17, :] = Bv

    tt = _make_T(H).T.copy()  # lhsT[k,m] = Tmat[m,k]
    dtm = _make_D(H).T.copy()

    w2t = np.asarray(W2, np.float32).T.astype(ml_dtypes.bfloat16)  # [128,16]

    b2t = np.tile(np.asarray(b2, np.float32)[None, :], (128, 32)).reshape(128, 512)

    return {
        "wext": wext,
        "tt": tt.astype(np.float32),
        "dt": dtm.astype(np.float32),
        "b1p": b1p.reshape(128, 1).astype(np.float32),
        "w2t": w2t,
        "b2t": b2t.astype(np.float32),
    }


_cached_module = None


def kernel(**inputs):
    global _cached_module
    from concourse import bass_utils

    x = np.asarray(inputs["x"], np.float32)
    consts = make_const_inputs(
        inputs["bias_x"], inputs["bias_y"], inputs["W1"],
        inputs["b1"], inputs["W2"], inputs["b2"])

    if _cached_module is None:
        _cached_module = build_module()
    nc = _cached_module

    in_maps = []
    for c in range(N_CORES):
        xs = np.ascontiguousarray(x[c * B_SHARD:(c + 1) * B_SHARD])
        m = {"x": xs, "xr": xs}
        m.update(consts)
        in_maps.append(m)

    res = bass_utils.run_bass_kernel_spmd(nc, in_maps, core_ids=list(range(N_CORES)))
    out = np.concatenate([res.results[c]["out"] for c in range(N_CORES)], axis=0)
    return out.astype(np.float32)


# revision 31
# speedup vs baseline: 1.0609x; 1.0609x over previous
"""Trainium2 Bass kernel for nn_NeuralCellularAutomata.

Reference computation (B=32, C=16, H=W=128):
    xsum = x.sum(ch)                                  (B,1,H,W)
    sx = conv(xsum, SOBEL_X) + bias_x[c]              (B,16,H,W)
    sy = conv(xsum, SOBEL_X.T) + bias_y[c]            (B,16,H,W)
    p  = concat([sx, sy, x]) transposed to (B,W,H,48)
    h  = relu(p @ W1.T + b1)                          (B,W,H,128)
    out = h @ W2.T + b2                               (B,W,H,16)

Device algebra (validated vs reference on host, rel err ~3e-7):
    h_pre[o,n] = sum_c W1[o,32+c] x[c,n] + A[o] u[n] + B[o] v[n] + b1'[o]
    with A = W1[:,0:16].sum(1), B = W1[:,16:32].sum(1),
    b1' = b1 + W1[:,0:16]@bias_x + W1[:,16:32]@bias_y,
    u = Tmat @ hdiff (vertical [1,2,1]/8 conv as matmul, horiz diff on DVE)
    v = Dmat @ hsmooth (vertical [-1,0,1]/8 as matmul, horiz [1,2,1] on DVE)

Sharding: pure data-parallel, 4 batches per core across 8 cores.

Per-core dataflow (per batch):
    xh [h,(c,w)] <- DMA; GPSIMD tree-sum -> xsum [h,w] (padded cols)
    DVE horiz diff/smooth; PE matmuls Tmat/Dmat -> u,v in PSUM
    flatten-DMA u,v into rhs rows 16,17 of the channel-major x tile
    MM1 (f32r, 2-way row-tiled K=18 at partitions 0/64) -> h_pre PSUM
    relu+bias: ACT + DVE split, writes h bf16
    MM2 flipped (lhsT=h block bf16, rhs=W2^T) -> out^T [w,(h,o)] PSUM
    DVE +b2 -> SBUF -> contiguous DMA store (no transpose needed)
"""

import numpy as np
import ml_dtypes

B_FULL, C, H, W = 32, 16, 128, 128
N_CORES = 8
B_SHARD = B_FULL // N_CORES  # 4
O1 = 128  # hidden
O2 = 16   # out channels
NSP = H * W  # spatial per batch = 16384


def _make_T(n):
    T = np.zeros((n, n), np.float32)
    for m in range(n):
        for dk, wt in ((-1, 1.0), (0, 2.0), (1, 1.0)):
            k = m + dk
            if 0 <= k < n:
                T[m, k] = wt / 8.0
    return T


def _make_D(n):
    D = np.zeros((n, n), np.float32)
    for m in range(n):
        for dk, wt in ((-1, -1.0), (1, 1.0)):
            k = m + dk
            if 0 <= k < n:
                D[m, k] = wt / 8.0
    return D


def build_module():
    import concourse.bacc as bacc
    import concourse.mybir as mybir
    import concourse.tile as tile

    f32 = mybir.dt.float32
    f32r = mybir.dt.float32r
    bf16 = mybir.dt.bfloat16

    nc = bacc.Bacc("TRN2", target_bir_lowering=False, debug=False)

    x_d = nc.dram_tensor("x", [B_SHARD, C, H, W], f32r, kind="ExternalInput").ap()
    xr_d = x_d
    wext_d = nc.dram_tensor("wext", [128, 128], f32r, kind="ExternalInput").ap()
    tt_d = nc.dram_tensor("tt", [128, 128], f32, kind="ExternalInput").ap()
    dt_d = nc.dram_tensor("dt", [128, 128], f32, kind="ExternalInput").ap()
    b1p_d = nc.dram_tensor("b1p", [128, 1], f32, kind="ExternalInput").ap()
    w2t_d = nc.dram_tensor("w2t", [128, O2], bf16, kind="ExternalInput").ap()
    b2t_d = nc.dram_tensor("b2t", [128, 512], f32, kind="ExternalInput").ap()
    zed_d = nc.dram_tensor("zed", [14, 64 * W], f32r, kind="ExternalInput").ap()
    out_d = nc.dram_tensor("out", [B_SHARD, W, H, O2], f32, kind="ExternalOutput").ap()

    RELU = mybir.ActivationFunctionType.Relu
    ADD = mybir.AluOpType.add
    SUB = mybir.AluOpType.subtract
    MULT = mybir.AluOpType.mult
    MAX = mybir.AluOpType.max

    with tile.TileContext(nc) as tc:
        with (
            tc.tile_pool(name="consts", bufs=1) as pc,
            tc.tile_pool(name="xh", bufs=2) as pxh,
            tc.tile_pool(name="xc", bufs=2) as pxc,
            tc.tile_pool(name="gpt", bufs=2) as pgp,
            tc.tile_pool(name="conv", bufs=2) as pcv,
            tc.tile_pool(name="hbuf", bufs=3) as ph,
            tc.tile_pool(name="outs", bufs=2) as pos,
            tc.tile_pool(name="mm1p", bufs=3, space="PSUM") as pmm1,
            tc.tile_pool(name="outp", bufs=2, space="PSUM") as pop,
        ):
            # constants
            wext = pc.tile([128, 128], f32r)
            ttm = pc.tile([128, 128], f32)
            dtm = pc.tile([128, 128], f32)
            b1p = pc.tile([128, 1], f32)
            w2t = pc.tile([128, O2], bf16)
            b2t = pc.tile([128, 512], f32)
            xcA = pc.tile([128, 64 * W], f32r)
            xcB = pc.tile([128, 64 * W], f32r)
            def emit_const_loads():
                nc.scalar.dma_start(ttm[:, :], tt_d)
                nc.scalar.dma_start(dtm[:, :], dt_d)
                nc.scalar.dma_start(wext[:, :], wext_d)
                nc.scalar.dma_start(b1p[:, :], b1p_d)
                nc.scalar.dma_start(w2t[:, :], w2t_d)
                nc.scalar.dma_start(b2t[:, :], b2t_d)
                for xct in (xcA, xcB):
                    nc.scalar.dma_start(xct[17:24, :], zed_d[0:7, :])
                    nc.scalar.dma_start(xct[81:88, :], zed_d[7:14, :])

            def emit_prologue(b):
                # load x (h-major); channel-sum -> padded xsum
                xh = pxh.tile([128, C * W], f32r, tag="xh")
                xh_dma = nc.sync.dma_start(
                    xh[:, :].rearrange("p (c w) -> p c w", c=C),
                    x_d[b].rearrange("c h w -> h c w"))
                xh3 = xh[:, :].rearrange("p (c w) -> p c w", c=C)

                xsp = pgp.tile([128, W + 2], f32, tag="xsp")
                if b == 0:
                    # startup: single DVE reduce (DVE idle, shortest latency)
                    nc.vector.memset(xsp[:, 0:1], 0.0)
                    nc.vector.memset(xsp[:, W + 1:W + 2], 0.0)
                    nc.vector.tensor_reduce(
                        xsp[:, 1:W + 1],
                        xh[:, :].rearrange("p (c w) -> p w c", c=C),
                        mybir.AxisListType.X, ADD)
                else:
                    # steady state: GPSIMD tree-sum (Pool has slack)
                    t1 = pgp.tile([128, 8 * W], f32, tag="t1")
                    t13 = t1[:, :].rearrange("p (c w) -> p c w", c=8)
                    t2 = pgp.tile([128, 4 * W], f32, tag="t2")
                    t23 = t2[:, :].rearrange("p (c w) -> p c w", c=4)
                    nc.gpsimd.tensor_tensor(t1[:, :], xh3[:, 0:8, :], xh3[:, 8:16, :], ADD)
                    nc.gpsimd.tensor_tensor(t2[:, :], t13[:, 0:4, :], t13[:, 4:8, :], ADD)
                    nc.gpsimd.tensor_tensor(t2[:, 0:2 * W], t23[:, 0:2, :], t23[:, 2:4, :], ADD)
                    nc.gpsimd.memset(xsp[:, 0:1], 0.0)
                    nc.gpsimd.memset(xsp[:, W + 1:W + 2], 0.0)
                    nc.gpsimd.tensor_tensor(xsp[:, 1:W + 1], t2[:, 0:W], t2[:, W:2 * W], ADD)

                # channel-major x tile (x rows load early; uv rows written later)
                xc = xcA if b % 2 == 0 else xcB
                xcs[b] = xc
                d1 = nc.sync.dma_start(
                    xc[0:C, :], xr_d[b, :, 0:64, :].rearrange("c h w -> c (h w)"))
                d2 = nc.sync.dma_start(
                    xc[64:64 + C, :], xr_d[b, :, 64:128, :].rearrange("c h w -> c (h w)"))
                del d1, d2

                # horizontal conv parts on DVE
                hdiff = pcv.tile([128, W], f32, tag="hdiff")
                hsm = pcv.tile([128, W], f32, tag="hsm")
                nc.vector.tensor_tensor(hdiff[:, :], xsp[:, 2:W + 2], xsp[:, 0:W], SUB)
                nc.vector.tensor_scalar(hsm[:, :], xsp[:, 1:W + 1], 2.0, None, MULT)
                nc.vector.tensor_tensor(hsm[:, :], hsm[:, :], xsp[:, 0:W], ADD)
                nc.vector.tensor_tensor(hsm[:, :], hsm[:, :], xsp[:, 2:W + 2], ADD)

                return hdiff, hsm

            def emit_prologue2(b, hdiff, hsm):
                # vertical conv parts as PE matmuls -> uv psum -> sbuf
                uvp = pop.tile([128, 2 * W], f32, tag="op")
                nc.tensor.matmul(uvp[:, 0:W], lhsT=ttm[:, :], rhs=hdiff[:, :])
                nc.tensor.matmul(uvp[:, W:2 * W], lhsT=dtm[:, :], rhs=hsm[:, :])
                uvs = pcv.tile([128, 2 * W], f32r, tag="uvs")
                nc.scalar.copy(uvs[:, :], uvp[:, :])

                # flatten u,v into rhs rows 16,17 of each group (SWDGE ring)
                xc = xcs[b]
                for g in range(2):
                    nc.scalar.dma_start(
                        xc[16 + 64 * g:17 + 64 * g, :].rearrange("p (h w) -> p h w", h=64),
                        uvs[64 * g:64 * g + 64, 0:W])
                    nc.scalar.dma_start(
                        xc[24 + 64 * g:25 + 64 * g, :].rearrange("p (h w) -> p h w", h=64),
                        uvs[64 * g:64 * g + 64, W:2 * W])

            def emit_main_pairs(b, xc, pairs):
                xcr = xc[:, :]
                wextr = wext[:, :]
                outs = outss[b]
                for pair in pairs:
                    # out psum accumulates 2 rounds = 16 G0 rows + 16 G1 rows
                    op = pop.tile([128, 512], f32, tag="op")
                    for rr in range(2):
                        r = pair * 2 + rr
                        mmA = pmm1.tile([128, 1024], f32, tag="mm1")
                        mmB = pmm1.tile([128, 1024], f32, tag="mm1")
                        for i in range(2):
                            nc.tensor.matmul(
                                mmA[:, 512 * i:512 * (i + 1)],
                                lhsT=wextr[0:25, :],
                                rhs=xcr[0:25, 1024 * r + 512 * i:1024 * r + 512 * (i + 1)])
                        for i in range(2):
                            nc.tensor.matmul(
                                mmB[:, 512 * i:512 * (i + 1)],
                                lhsT=wextr[64:89, :],
                                rhs=xcr[64:89, 1024 * r + 512 * i:1024 * r + 512 * (i + 1)])

                        # relu + bias -> h (bf16): whole-tile ACT/DVE assignment
                        # (10 of 16 tiles per batch on ACT, 6 on DVE)
                        h = ph.tile([128, 2048], bf16, tag="h")
                        for ti, mm in ((0, mmA), (1, mmB)):
                            idx = 2 * r + ti
                            if idx % 8 in (0, 2, 3, 5, 6):
                                nc.scalar.activation(
                                    h[:, 1024 * ti:1024 * (ti + 1)], mm[:, :],
                                    RELU, bias=b1p[:, :])
                            else:
                                nc.vector.tensor_scalar(
                                    h[:, 1024 * ti:1024 * (ti + 1)], mm[:, :],
                                    b1p[:, :], 0.0, ADD, MAX)

                        # flipped MM2: 16 blocks of 128 spatial
                        # op layout: [G0 rows 16p..16p+16]*16 | [G1 rows]*16
                        for q in range(16):
                            g = q // 8
                            qq = q % 8
                            hrow = 64 * g + 8 * r + qq
                            off = 256 * g + (hrow % 16) * O2
                            nc.tensor.matmul(
                                op[:, off:off + O2],
                                lhsT=h[:, 1024 * g + 128 * qq:1024 * g + 128 * (qq + 1)],
                                rhs=w2t[:, :])

                    # drain out psum: +b2 -> outs (col layout: (pair, g, j, o))
                    nc.vector.tensor_tensor(
                        outs[:, pair * 512:(pair + 1) * 512], op[:, :], b2t[:, :], ADD)

            def emit_store(b):
                outs = outss[b]
                # store: outs [w, (pair, g, j, o)] -> DRAM out[b, w, h, o]; h = 64g+16p+j
                for g in range(2):
                    nc.sync.dma_start(
                        out_d[b, :, 64 * g:64 * (g + 1), :].rearrange(
                            "w (pr j) o -> w pr j o", pr=4),
                        outs[:, :].rearrange(
                            "w (pr gg jo) -> w pr gg jo", pr=4, gg=2)[:, :, g, :])

            # software pipeline, fine-grained:
            #   part1(b+1) | main(b) pairs 0,1 | part2(b+1) | main(b) pairs 2,3
            xcs = {}
            convs = {}
            outss = {}
            for b in range(B_SHARD):
                outs_tile = pos.tile([128, H * O2], f32, tag="outs")
                outss[b] = outs_tile
            emit_const_loads()
            convs[0] = emit_prologue(0)
            emit_prologue2(0, *convs.pop(0))
            for b in range(B_SHARD):
                if b + 1 < B_SHARD:
                    convs[b + 1] = emit_prologue(b + 1)
                emit_main_pairs(b, xcs[b], [0, 1])
                if b + 1 < B_SHARD:
                    emit_prologue2(b + 1, *convs.pop(b + 1))
                emit_main_pairs(b, xcs[b], [2, 3])
                emit_store(b)
                del xcs[b]

    nc.compile()
    return nc


def make_const_inputs(bias_x, bias_y, W1, b1, W2, b2):
    W1 = np.asarray(W1, np.float32)
    A = W1[:, 0:16].sum(axis=1)
    Bv = W1[:, 16:32].sum(axis=1)
    W1x = W1[:, 32:48]
    b1p = (np.asarray(b1, np.float32)
           + W1[:, 0:16] @ np.asarray(bias_x, np.float32)
           + W1[:, 16:32] @ np.asarray(bias_y, np.float32))

    wext = np.zeros((128, 128), np.float32)
    for g in range(2):
        base = 64 * g
        wext[base:base + 16, :] = W1x.T  # row c -> W1x[:, c]
        wext[base + 16, :] = A
        wext[base + 24, :] = Bv

    tt = _make_T(H).T.copy()  # lhsT[k,m] = Tmat[m,k]
    dtm = _make_D(H).T.copy()

    w2t = np.asarray(W2, np.float32).T.astype(ml_dtypes.bfloat16)  # [128,16]

    b2t = np.tile(np.asarray(b2, np.float32)[None, :], (128, 32)).reshape(128, 512)

    return {
        "zed": np.zeros((14, 8192), np.float32),
        "wext": wext,
        "tt": tt.astype(np.float32),
        "dt": dtm.astype(np.float32),
        "b1p": b1p.reshape(128, 1).astype(np.float32),
        "w2t": w2t,
        "b2t": b2t.astype(np.float32),
    }


_cached_module = None


def kernel(**inputs):
    global _cached_module
    from concourse import bass_utils

    x = np.asarray(inputs["x"], np.float32)
    consts = make_const_inputs(
        inputs["bias_x"], inputs["bias_y"], inputs["W1"],
        inputs["b1"], inputs["W2"], inputs["b2"])

    if _cached_module is None:
        _cached_module = build_module()
    nc = _cached_module

    in_maps = []
    for c in range(N_CORES):
        xs = np.ascontiguousarray(x[c * B_SHARD:(c + 1) * B_SHARD])
        m = {"x": xs}
        m.update(consts)
        in_maps.append(m)

    res = bass_utils.run_bass_kernel_spmd(nc, in_maps, core_ids=list(range(N_CORES)))
    out = np.concatenate([res.results[c]["out"] for c in range(N_CORES)], axis=0)
    return out.astype(np.float32)


# revision 46
# speedup vs baseline: 131.3848x; 123.8453x over previous
"""Trainium2 Bass kernel for nn_NeuralCellularAutomata.

Reference computation (B=32, C=16, H=W=128):
    xsum = x.sum(ch)                                  (B,1,H,W)
    sx = conv(xsum, SOBEL_X) + bias_x[c]              (B,16,H,W)
    sy = conv(xsum, SOBEL_X.T) + bias_y[c]            (B,16,H,W)
    p  = concat([sx, sy, x]) transposed to (B,W,H,48)
    h  = relu(p @ W1.T + b1)                          (B,W,H,128)
    out = h @ W2.T + b2                               (B,W,H,16)

Device algebra (validated vs reference on host, rel err ~3e-7):
    h_pre[o,n] = sum_c W1[o,32+c] x[c,n] + A[o] u[n] + B[o] v[n] + b1'[o]
    with A = W1[:,0:16].sum(1), B = W1[:,16:32].sum(1),
    b1' = b1 + W1[:,0:16]@bias_x + W1[:,16:32]@bias_y,
    u = Tmat @ hdiff (vertical [1,2,1]/8 conv as matmul, horiz diff on DVE)
    v = Dmat @ hsmooth (vertical [-1,0,1]/8 as matmul, horiz [1,2,1] on DVE)

Sharding: pure data-parallel, 4 batches per core across 8 cores.

Per-core dataflow (per batch), software-pipelined across batches
(xh/xsum/conv two ahead, xc loads + uv/flatten one ahead):
    xh [h,(c,w)] <- split DMA; GPSIMD tree-sum -> xsum [h,w] (padded cols;
      batch 0 uses one DVE tensor_reduce for lowest startup latency)
    DVE horiz diff/smooth; PE matmuls Tmat/Dmat -> u,v in PSUM
    flatten-DMA u,v into rhs rows 16/24 (distinct SBUF DMA ports) of the
      persistent ping-pong channel-major x tiles; gap rows 17..23 are
      zero-filled once so the K=25 matmul reads no garbage
    MM1 (f32r, 2-way row-tiled K=25 at partitions 0/64) -> h_pre PSUM
    relu+bias: whole [128,1024]-tile ops, interleaved 10:6 ACT:DVE,
      writes h bf16
    MM2 flipped (lhsT=h block bf16 streamed as stationary, rhs=W2^T
      [128,16]) -> out^T [w,(h,o)] PSUM - output needs no transpose
    DVE +b2 -> SBUF -> contiguous per-pair DMA stores

Cost-model timeline: ~80 us/core; HW rel err vs fp32 reference ~3.5e-3.
"""

import numpy as np
import ml_dtypes

B_FULL, C, H, W = 32, 16, 128, 128
N_CORES = 8
B_SHARD = B_FULL // N_CORES  # 4
O1 = 128  # hidden
O2 = 16   # out channels
NSP = H * W  # spatial per batch = 16384


def _make_T(n):
    T = np.zeros((n, n), np.float32)
    for m in range(n):
        for dk, wt in ((-1, 1.0), (0, 2.0), (1, 1.0)):
            k = m + dk
            if 0 <= k < n:
                T[m, k] = wt / 8.0
    return T


def _make_D(n):
    D = np.zeros((n, n), np.float32)
    for m in range(n):
        for dk, wt in ((-1, -1.0), (1, 1.0)):
            k = m + dk
            if 0 <= k < n:
                D[m, k] = wt / 8.0
    return D


def build_module():
    import concourse.bacc as bacc
    import concourse.mybir as mybir
    import concourse.tile as tile

    f32 = mybir.dt.float32
    f32r = mybir.dt.float32r
    bf16 = mybir.dt.bfloat16

    nc = bacc.Bacc("TRN2", target_bir_lowering=False, debug=False)

    x_d = nc.dram_tensor("x", [B_SHARD, C, H, W], f32r, kind="ExternalInput").ap()
    xr_d = x_d
    wext_d = nc.dram_tensor("wext", [128, 128], f32r, kind="ExternalInput").ap()
    tt_d = nc.dram_tensor("tt", [128, 128], f32, kind="ExternalInput").ap()
    dt_d = nc.dram_tensor("dt", [128, 128], f32, kind="ExternalInput").ap()
    b1p_d = nc.dram_tensor("b1p", [128, 1], f32, kind="ExternalInput").ap()
    w2t_d = nc.dram_tensor("w2t", [128, O2], bf16, kind="ExternalInput").ap()
    b2t_d = nc.dram_tensor("b2t", [128, 512], f32, kind="ExternalInput").ap()
    zed_d = nc.dram_tensor("zed", [14, 64 * W], f32r, kind="ExternalInput").ap()
    out_d = nc.dram_tensor("out", [B_SHARD, W, H, O2], f32, kind="ExternalOutput").ap()

    RELU = mybir.ActivationFunctionType.Relu
    ADD = mybir.AluOpType.add
    SUB = mybir.AluOpType.subtract
    MULT = mybir.AluOpType.mult
    MAX = mybir.AluOpType.max

    with tile.TileContext(nc) as tc:
        with (
            tc.tile_pool(name="consts", bufs=1) as pc,
            tc.tile_pool(name="xh", bufs=3) as pxh,
            tc.tile_pool(name="xc", bufs=2) as pxc,
            tc.tile_pool(name="gpt", bufs=3) as pgp,
            tc.tile_pool(name="conv", bufs=3) as pcv,
            tc.tile_pool(name="hbuf", bufs=6) as ph,
            tc.tile_pool(name="outs", bufs=2) as pos,
            tc.tile_pool(name="mm1p", bufs=3, space="PSUM") as pmm1,
            tc.tile_pool(name="outp", bufs=2, space="PSUM") as pop,
        ):
            # constants
            wext = pc.tile([128, 128], f32r)
            ttm = pc.tile([128, 128], f32)
            dtm = pc.tile([128, 128], f32)
            b1p = pc.tile([128, 1], f32)
            w2t = pc.tile([128, O2], bf16)
            b2t = pc.tile([128, 512], f32)
            xcA = pc.tile([128, 64 * W], f32r)
            xcB = pc.tile([128, 64 * W], f32r)
            def emit_const_loads():
                nc.scalar.dma_start(ttm[:, :], tt_d)
                nc.scalar.dma_start(dtm[:, :], dt_d)
                nc.scalar.dma_start(wext[:, :], wext_d)
                nc.scalar.dma_start(b1p[:, :], b1p_d)
                nc.scalar.dma_start(w2t[:, :], w2t_d)
                nc.scalar.dma_start(b2t[:, :], b2t_d)
                for xct in (xcA, xcB):
                    nc.scalar.dma_start(xct[17:24, :], zed_d[0:7, :])
                    nc.scalar.dma_start(xct[81:88, :], zed_d[7:14, :])

            def emit_prologue(b):
                # load x (h-major); channel-sum -> padded xsum
                xh = pxh.tile([128, C * W], f32r, tag="xh")
                nc.sync.dma_start(
                    xh[:, 0:8 * W].rearrange("p (c w) -> p c w", c=8),
                    x_d[b, 0:8].rearrange("c h w -> h c w"))
                nc.sync.dma_start(
                    xh[:, 8 * W:].rearrange("p (c w) -> p c w", c=8),
                    x_d[b, 8:16].rearrange("c h w -> h c w"))
                xh3 = xh[:, :].rearrange("p (c w) -> p c w", c=C)

                xsp = pgp.tile([128, W + 2], f32, tag="xsp")
                if b == 0:
                    # startup: single DVE reduce (DVE idle, shortest latency)
                    nc.vector.memset(xsp[:, 0:1], 0.0)
                    nc.vector.memset(xsp[:, W + 1:W + 2], 0.0)
                    nc.vector.tensor_reduce(
                        xsp[:, 1:W + 1],
                        xh[:, :].rearrange("p (c w) -> p w c", c=C),
                        mybir.AxisListType.X, ADD)
                else:
                    # steady state: GPSIMD tree-sum (Pool has slack)
                    t1 = pgp.tile([128, 8 * W], f32, tag="t1")
                    t13 = t1[:, :].rearrange("p (c w) -> p c w", c=8)
                    # halves pipelined against the split xh load
                    nc.gpsimd.tensor_tensor(t1[:, 0:4 * W], xh3[:, 0:4, :], xh3[:, 4:8, :], ADD)
                    nc.gpsimd.tensor_tensor(t1[:, 4 * W:], xh3[:, 8:12, :], xh3[:, 12:16, :], ADD)
                    nc.gpsimd.tensor_tensor(t1[:, 0:4 * W], t13[:, 0:4, :], t13[:, 4:8, :], ADD)
                    nc.gpsimd.tensor_tensor(t1[:, 0:2 * W], t13[:, 0:2, :], t13[:, 2:4, :], ADD)
                    nc.gpsimd.memset(xsp[:, 0:1], 0.0)
                    nc.gpsimd.memset(xsp[:, W + 1:W + 2], 0.0)
                    nc.gpsimd.tensor_tensor(xsp[:, 1:W + 1], t1[:, 0:W], t1[:, W:2 * W], ADD)

                # horizontal conv parts on DVE
                hdiff = pcv.tile([128, W], f32, tag="hdiff")
                hsm = pcv.tile([128, W], f32, tag="hsm")
                nc.vector.tensor_tensor(hdiff[:, :], xsp[:, 2:W + 2], xsp[:, 0:W], SUB)
                nc.vector.tensor_scalar(hsm[:, :], xsp[:, 1:W + 1], 2.0, None, MULT)
                nc.vector.tensor_tensor(hsm[:, :], hsm[:, :], xsp[:, 0:W], ADD)
                nc.vector.tensor_tensor(hsm[:, :], hsm[:, :], xsp[:, 2:W + 2], ADD)

                return hdiff, hsm

            def emit_xc_loads(b):
                xc = xcA if b % 2 == 0 else xcB
                xcs[b] = xc
                nc.sync.dma_start(
                    xc[0:C, :], xr_d[b, :, 0:64, :].rearrange("c h w -> c (h w)"))
                nc.sync.dma_start(
                    xc[64:64 + C, :], xr_d[b, :, 64:128, :].rearrange("c h w -> c (h w)"))

            def emit_prologue2(b, hdiff, hsm):
                xc = xcs[b]
                # vertical conv parts as PE matmuls -> uv psum -> sbuf
                uvp = pop.tile([128, 2 * W], f32, tag="op")
                nc.tensor.matmul(uvp[:, 0:W], lhsT=ttm[:, :], rhs=hdiff[:, :])
                nc.tensor.matmul(uvp[:, W:2 * W], lhsT=dtm[:, :], rhs=hsm[:, :])
                uvs = pcv.tile([128, 2 * W], f32r, tag="uvs")
                nc.scalar.copy(uvs[:, :], uvp[:, :])

                # flatten u (row 16, ACT ring) and v (row 24, SP ring)
                for g in range(2):
                    nc.scalar.dma_start(
                        xc[16 + 64 * g:17 + 64 * g, :].rearrange("p (h w) -> p h w", h=64),
                        uvs[64 * g:64 * g + 64, 0:W])
                    nc.sync.dma_start(
                        xc[24 + 64 * g:25 + 64 * g, :].rearrange("p (h w) -> p h w", h=64),
                        uvs[64 * g:64 * g + 64, W:2 * W])

            def emit_main_pairs(b, xc, pairs):
                xcr = xc[:, :]
                wextr = wext[:, :]
                outs = outss[b]
                for pair in pairs:
                    # out psum accumulates 2 rounds = 16 G0 rows + 16 G1 rows
                    op = pop.tile([128, 512], f32, tag="op")
                    for rr in range(2):
                        r = pair * 2 + rr
                        mmA = pmm1.tile([128, 1024], f32, tag="mm1")
                        mmB = pmm1.tile([128, 1024], f32, tag="mm1")
                        for i in range(2):
                            nc.tensor.matmul(
                                mmA[:, 512 * i:512 * (i + 1)],
                                lhsT=wextr[0:25, :],
                                rhs=xcr[0:25, 1024 * r + 512 * i:1024 * r + 512 * (i + 1)])
                        for i in range(2):
                            nc.tensor.matmul(
                                mmB[:, 512 * i:512 * (i + 1)],
                                lhsT=wextr[64:89, :],
                                rhs=xcr[64:89, 1024 * r + 512 * i:1024 * r + 512 * (i + 1)])

                        # relu + bias -> h (bf16): whole-tile ACT/DVE assignment
                        # (10 of 16 tiles per batch on ACT, 6 on DVE)
                        h = ph.tile([128, 2048], bf16, tag="h")
                        for ti, mm in ((0, mmA), (1, mmB)):
                            idx = 2 * r + ti
                            if idx % 8 in (0, 2, 3, 5, 6):
                                nc.scalar.activation(
                                    h[:, 1024 * ti:1024 * (ti + 1)], mm[:, :],
                                    RELU, bias=b1p[:, :])
                            else:
                                nc.vector.tensor_scalar(
                                    h[:, 1024 * ti:1024 * (ti + 1)], mm[:, :],
                                    b1p[:, :], 0.0, ADD, MAX)

                        # flipped MM2: 16 blocks of 128 spatial
                        # op layout: [G0 rows 16p..16p+16]*16 | [G1 rows]*16
                        for q in range(16):
                            g = q // 8
                            qq = q % 8
                            hrow = 64 * g + 8 * r + qq
                            off = 256 * g + (hrow % 16) * O2
                            nc.tensor.matmul(
                                op[:, off:off + O2],
                                lhsT=h[:, 1024 * g + 128 * qq:1024 * g + 128 * (qq + 1)],
                                rhs=w2t[:, :])

                    # drain out psum: +b2 -> outs (col layout: (pair, g, j, o))
                    nc.vector.tensor_tensor(
                        outs[:, pair * 512:(pair + 1) * 512], op[:, :], b2t[:, :], ADD)
                    # store this pair: h = 64g + 16*pair + j
                    for g in range(2):
                        nc.sync.dma_start(
                            out_d[b, :, 64 * g + 16 * pair:64 * g + 16 * (pair + 1), :],
                            outs[:, :].rearrange(
                                "w (pr gg j o) -> w pr gg j o",
                                pr=4, gg=2, j=16)[:, pair, g])


            # software pipeline, fine-grained:
            #   part1(b+1) | main(b) pairs 0,1 | part2(b+1) | main(b) pairs 2,3
            xcs = {}
            convs = {}
            outss = {}
            for b in range(B_SHARD):
                outs_tile = pos.tile([128, H * O2], f32, tag="outs")
                outss[b] = outs_tile
            emit_const_loads()
            convs[0] = emit_prologue(0)
            emit_xc_loads(0)
            emit_prologue2(0, *convs.pop(0))
            convs[1] = emit_prologue(1)
            for b in range(B_SHARD):
                if b + 1 < B_SHARD:
                    emit_xc_loads(b + 1)
                if b + 2 < B_SHARD:
                    convs[b + 2] = emit_prologue(b + 2)
                emit_main_pairs(b, xcs[b], [0, 1])
                if b + 1 < B_SHARD:
                    emit_prologue2(b + 1, *convs.pop(b + 1))
                emit_main_pairs(b, xcs[b], [2, 3])
                del xcs[b]

    nc.compile()
    return nc


def make_const_inputs(bias_x, bias_y, W1, b1, W2, b2):
    W1 = np.asarray(W1, np.float32)
    A = W1[:, 0:16].sum(axis=1)
    Bv = W1[:, 16:32].sum(axis=1)
    W1x = W1[:, 32:48]
    b1p = (np.asarray(b1, np.float32)
           + W1[:, 0:16] @ np.asarray(bias_x, np.float32)
           + W1[:, 16:32] @ np.asarray(bias_y, np.float32))

    wext = np.zeros((128, 128), np.float32)
    for g in range(2):
        base = 64 * g
        wext[base:base + 16, :] = W1x.T  # row c -> W1x[:, c]
        wext[base + 16, :] = A
        wext[base + 24, :] = Bv

    tt = _make_T(H).T.copy()  # lhsT[k,m] = Tmat[m,k]
    dtm = _make_D(H).T.copy()

    w2t = np.asarray(W2, np.float32).T.astype(ml_dtypes.bfloat16)  # [128,16]

    b2t = np.tile(np.asarray(b2, np.float32)[None, :], (128, 32)).reshape(128, 512)

    return {
        "zed": np.zeros((14, 8192), np.float32),
        "wext": wext,
        "tt": tt.astype(np.float32),
        "dt": dtm.astype(np.float32),
        "b1p": b1p.reshape(128, 1).astype(np.float32),
        "w2t": w2t,
        "b2t": b2t.astype(np.float32),
    }


_cached_module = None


def kernel(**inputs):
    global _cached_module
    from concourse import bass_utils

    x = np.asarray(inputs["x"], np.float32)
    consts = make_const_inputs(
        inputs["bias_x"], inputs["bias_y"], inputs["W1"],
        inputs["b1"], inputs["W2"], inputs["b2"])

    if _cached_module is None:
        _cached_module = build_module()
    nc = _cached_module

    in_maps = []
    for c in range(N_CORES):
        xs = np.ascontiguousarray(x[c * B_SHARD:(c + 1) * B_SHARD])
        m = {"x": xs}
        m.update(consts)
        in_maps.append(m)

    last_err = None
    for attempt in range(3):
        try:
            res = bass_utils.run_bass_kernel_spmd(
                nc, in_maps, core_ids=list(range(N_CORES)))
            break
        except Exception as e:  # transient NRT_EXEC_UNIT_UNRECOVERABLE etc.
            last_err = e
            _cached_module = None
            nc = _cached_module = build_module()
    else:
        raise last_err
    out = np.concatenate([res.results[c]["out"] for c in range(N_CORES)], axis=0)
    return out.astype(np.float32)
